# revision 3
# baseline (speedup 1.0000x reference)
"""Longformer-style sparse attention block (nn_BasicNetwork_22892175688067).

Full-input contract: kernel(**inputs) takes the unsharded inputs and returns
the full [B, S, D] fp32 output.  Internally the (batch, head) grid is sharded
across 8 NeuronCores: core = b*4 + hg owns batch b and heads [3*hg, 3*hg+3).
Each core:
  - gathers x[b] = emb[ids[b]] on-device (indirect DMA),
  - projects q/k/v (+ global-token projections) for its 3 heads in bf16,
  - runs banded window attention + global tokens,
  - applies the output projection slice -> a partial [S, D] fp32 output.
The host sums the 4 per-batch partials (tensor-parallel unshard).
"""

import os
import sys
import types

import numpy as np

for _p in ("/opt/trn_rl_repo",):
    if os.path.isdir(_p) and _p not in sys.path:
        sys.path.insert(0, _p)

import ml_dtypes

BF16 = ml_dtypes.bfloat16

B, S, D = 2, 4096, 768
H, DH, W, G, VOCAB = 12, 64, 256, 16, 50265
HPC = 3               # heads per core
C = S // W            # 16 chunks
QW = W                # queries per chunk (= one-sided window)
P = 128
FT = D // P           # 6 feature k-tiles
TT = S // P           # 32 token tiles
NT = S // 512         # 8 n-tiles for qk projection
NCORES = 8
SCALE = 1.0 / 8.0     # 1/sqrt(DH), exact power of two

_COMPILED = {}


def _chunk_schedule(c):
    """Per q-chunk list of (window j-tile, mask id or None).

    mask ids: 0=M_A (j>=i), 1=M_A1 (j>=i & j>=16), 2=M_B (128+j>=i),
              3=M_C (j<=i), 4=M_D (j<=i-128), 5=M_E (j>=16)
    """
    if c == 0:
        return [(2, 5), (3, None), (4, 3), (5, 4)]
    if c == 1:
        return [(0, 1), (1, 2), (2, None), (3, None), (4, 3), (5, 4)]
    if c == C - 1:
        return [(0, 0), (1, 2), (2, None), (3, None)]
    return [(0, 0), (1, 2), (2, None), (3, None), (4, 3), (5, 4)]


def _build_masks():
    j = np.arange(P)[:, None]
    i = np.arange(QW)[None, :]
    masks = np.zeros((6, P, QW), np.float32)
    masks[0] = (j >= i)
    masks[1] = (j >= i) & (j >= G)
    masks[2] = (P + j >= i)
    masks[3] = (j <= i)
    masks[4] = (j <= i - P)
    masks[5] = (j >= G) * np.ones((1, QW))
    return masks.astype(BF16)


def _install_axon_hooks():
    """Provide antenv.axon_hooks (missing in this image) so NTFF tracing works."""
    if "antenv.axon_hooks" in sys.modules:
        return
    mod = types.ModuleType("antenv.axon_hooks")
    hook = [None]
    mod.set_axon_ntff_profile_hook = lambda h: hook.__setitem__(0, h)
    mod.get_axon_ntff_profile_hook = lambda: hook[0]
    sys.modules["antenv.axon_hooks"] = mod
    try:
        import antenv

        antenv.axon_hooks = mod
        from trn_agent_boot.trn_boot import _ntff_profile_via_ctypes

        mod.set_axon_ntff_profile_hook(
            _ntff_profile_via_ctypes("/opt/axon/libaxon_pjrt.so")
        )
    except Exception:
        pass


def _patch_tile_drain():
    """This walrus build allows only ONE sync-wait per instruction.

    Split extra waits onto same-engine NoOps emitted just before the
    instruction (engines execute their stream in order, so chained
    single-wait nops are semantically identical to one multi-wait inst).
    """
    import concourse.mybir as mybir
    import concourse.tile as tile
    from concourse.vector_clock import ScopedClock

    if getattr(tile.TileContext, "_drain_split_patched", False):
        return

    _orig_add = tile.TileContext._add_instruction
    counter = [0]

    def _add_instruction(self, inst):
        si = getattr(inst, "sync_info", None)
        if si is not None and si.on_wait is not None and len(si.on_wait) > 1:
            waits = list(si.on_wait)
            for w in waits[:-1]:
                counter[0] += 1
                nop = mybir.InstNoOp(
                    name=f"WS-{counter[0]}", text_hint="wait_split"
                )
                nop.engine = inst.engine
                nop.sync_info = mybir.SyncInfo(on_wait=[w], on_update=[])
                _orig_add(self, nop)
            inst.sync_info = mybir.SyncInfo(
                on_wait=[waits[-1]], on_update=list(si.on_update)
            )
        _orig_add(self, inst)

    tile.TileContext._add_instruction = _add_instruction

    def _drain_and_barrier(self, tick_clock, wait_clock):
        drain1 = self.nc.sync.drain()
        wait_clock.add_sem_waits(
            drain1.ins, ScopedClock({None: tick_clock.global_clock})
        )
        si = drain1.ins.sync_info
        waits = list(si.on_wait) if si is not None and si.on_wait else []
        if len(waits) > 1:
            drain1.ins.sync_info = mybir.SyncInfo(
                on_wait=waits[:1], on_update=list(si.on_update)
            )
            for wchunk in waits[1:]:
                d = self.nc.sync.drain()
                d.ins.sync_info = mybir.SyncInfo(on_wait=[wchunk], on_update=[])
        self.nc.all_engine_barrier()
        assert self.sems is not None
        popped = self.nc._tile_sem_poison_stack.pop()
        assert popped is self._sem_poison
        self.nc.clear_and_free_semaphores(list(self.sems.allocated().values()))
        self.nc.all_engine_barrier()

    tile.TileContext._drain_and_barrier = _drain_and_barrier
    tile.TileContext._drain_split_patched = True


def build_nc():
    """Build the per-core Bass program (identical on all 8 cores)."""
    import concourse.bass as bass
    import concourse.mybir as mybir
    import concourse.tile as tile

    _patch_tile_drain()

    f32 = mybir.dt.float32
    bf16 = mybir.dt.bfloat16
    i32 = mybir.dt.int32
    AF = mybir.ActivationFunctionType
    OP = mybir.AluOpType

    nc = bass.Bass("TRN2", num_devices=NCORES)

    ids = nc.dram_tensor("ids", [S, 1], i32, kind="ExternalInput")
    emb = nc.dram_tensor("emb", [VOCAB, D], f32, kind="ExternalInput")
    wqk = nc.dram_tensor("wqk", [D, 768], bf16, kind="ExternalInput")
    bqk = nc.dram_tensor("bqk", [768], f32, kind="ExternalInput")
    wv = nc.dram_tensor("wv", [D, 384], bf16, kind="ExternalInput")
    bv = nc.dram_tensor("bv", [384], f32, kind="ExternalInput")
    wo0 = nc.dram_tensor("wo0", [P, D], bf16, kind="ExternalInput")
    wo1 = nc.dram_tensor("wo1", [DH, D], bf16, kind="ExternalInput")
    bo = nc.dram_tensor("bo", [D], f32, kind="ExternalInput")
    msk = nc.dram_tensor("msk", [6, P, QW], bf16, kind="ExternalInput")
    outp = nc.dram_tensor("out", [S, D], f32, kind="ExternalOutput")

    with tile.TileContext(nc) as tc:
        from contextlib import ExitStack

        with ExitStack() as ctx:
            pers = ctx.enter_context(tc.tile_pool(name="pers", bufs=1))

            # ---------------- persistent SBUF tensors ----------------
            wqk_sb = pers.tile([P, FT, 768], bf16, tag="wqk")
            bqk_sb = pers.tile([P, FT], f32, tag="bqk")
            wv_sb = pers.tile([P, FT, 384], bf16, tag="wv")
            wo0_sb = pers.tile([P, D], bf16, tag="wo0")
            wo1_sb = pers.tile([DH, D], bf16, tag="wo1")
            msk_sb = pers.tile([P, 6, QW], bf16, tag="msk")
            ones_sb = pers.tile([P, P], f32, tag="ones")
            bv1_sb = pers.tile([1, 384], f32, tag="bv1")
            bo1_sb = pers.tile([1, D], f32, tag="bo1")
            bvb_sb = pers.tile([P, 384], f32, tag="bvb")
            bob_sb = pers.tile([P, D], f32, tag="bob")
            # projection outputs
            blk_sb = pers.tile([P, 5, S], bf16, tag="blk")
            qg5_sb = pers.tile([P, P], bf16, tag="qg5")
            v_sb = pers.tile([P, TT, 2 * HPC, DH + 1], bf16, tag="vsb")
            ctx01_sb = pers.tile([P, S], bf16, tag="ctx01")
            ctx2_sb = pers.tile([DH, S], bf16, tag="ctx2")
            ctxh1_sb = pers.tile([DH, S], bf16, tag="ctxh1")
            pg_sb = pers.tile([P, TT, HPC * G], bf16, tag="pgsb")

            # ---------------- constant loads ----------------
            nc.sync.dma_start(
                out=wqk_sb[:], in_=wqk.ap().rearrange("(kt p) c -> p kt c", p=P)
            )
            nc.sync.dma_start(
                out=bqk_sb[:], in_=bqk.ap().rearrange("(kt p) -> p kt", p=P)
            )
            nc.sync.dma_start(
                out=wv_sb[:], in_=wv.ap().rearrange("(kt p) c -> p kt c", p=P)
            )
            nc.sync.dma_start(out=wo0_sb[:], in_=wo0.ap())
            nc.sync.dma_start(out=wo1_sb[:], in_=wo1.ap())
            nc.sync.dma_start(out=msk_sb[:], in_=msk.ap().rearrange("m p i -> p m i"))
            nc.sync.dma_start(out=bv1_sb[:], in_=bv.ap()[None, :])
            nc.sync.dma_start(out=bo1_sb[:], in_=bo.ap()[None, :])
            nc.gpsimd.memset(ones_sb[:], 1.0)
            nc.gpsimd.memset(v_sb[:, :, :, DH], 1.0)

            # broadcast biases across partitions via ones-matmul (fp32)
            with tc.tile_pool(name="bcast_ps", bufs=1, space="PSUM") as bps:
                bvp = bps.tile([P, 384], f32, tag="bvp")
                nc.tensor.matmul(
                    out=bvp[:], lhsT=ones_sb[0:1, :], rhs=bv1_sb[:],
                    start=True, stop=True,
                )
                nc.vector.tensor_copy(out=bvb_sb[:], in_=bvp[:])
                bop = bps.tile([P, D], f32, tag="bop")
                nc.tensor.matmul(
                    out=bop[:, 0:512], lhsT=ones_sb[0:1, :], rhs=bo1_sb[:, 0:512],
                    start=True, stop=True,
                )
                nc.tensor.matmul(
                    out=bop[:, 512:768], lhsT=ones_sb[0:1, :], rhs=bo1_sb[:, 512:768],
                    start=True, stop=True,
                )
                nc.vector.tensor_copy(out=bob_sb[:], in_=bop[:])

            # ---------------- gather + transpose + projections ----------------
            with tc.tile_pool(name="xt_pool", bufs=1) as xtp, \
                 tc.tile_pool(name="gather", bufs=3) as gp, \
                 tc.tile_pool(name="proj_ps", bufs=3, space="PSUM") as pps:
                xT_sb = xtp.tile([P, FT, S], bf16, tag="xT")

                for tt in range(TT):
                    idx_t = gp.tile([P, 1], i32, tag="idx")
                    nc.sync.dma_start(
                        out=idx_t[:], in_=ids.ap()[tt * P : (tt + 1) * P, :]
                    )
                    xg = gp.tile([P, D], f32, tag="xg")
                    nc.gpsimd.indirect_dma_start(
                        out=xg[:],
                        out_offset=None,
                        in_=emb.ap(),
                        in_offset=bass.IndirectOffsetOnAxis(ap=idx_t[:, :1], axis=0),
                    )
                    xb = gp.tile([P, D], bf16, tag="xb")
                    nc.scalar.copy(out=xb[:], in_=xg[:])
                    for ft in range(FT):
                        nc.scalar.dma_start(
                            out=xT_sb[:, ft, tt * P : (tt + 1) * P],
                            in_=xb[:, ft * P : (ft + 1) * P],
                            transpose=True,
                        )

                # q/k/qg/kg projections (feature-major outputs).
                # block cols: 0:(q0|q1) 1:(k0|k1) 2:(q2|qg2) 3:(k2|kg2) 4:(kg0|kg1)
                # B5 (qg0|qg1) done over token-tile 0 only -> qg5_sb
                for n in range(NT):
                    for bi in range(5):
                        ps = pps.tile([P, 512], f32, tag="pps")
                        for kt in range(FT):
                            nc.tensor.matmul(
                                out=ps[:],
                                lhsT=wqk_sb[:, kt, bi * P : (bi + 1) * P],
                                rhs=xT_sb[:, kt, n * 512 : (n + 1) * 512],
                                start=(kt == 0),
                                stop=(kt == FT - 1),
                            )
                        nc.vector.tensor_scalar_add(
                            out=blk_sb[:, bi, n * 512 : (n + 1) * 512],
                            in0=ps[:],
                            scalar1=bqk_sb[:, bi : bi + 1],
                        )
                ps5 = pps.tile([P, 512], f32, tag="pps")
                for kt in range(FT):
                    nc.tensor.matmul(
                        out=ps5[:, 0:P],
                        lhsT=wqk_sb[:, kt, 5 * P : 6 * P],
                        rhs=xT_sb[:, kt, 0:P],
                        start=(kt == 0),
                        stop=(kt == FT - 1),
                    )
                nc.vector.tensor_scalar_add(
                    out=qg5_sb[:], in0=ps5[:, 0:P], scalar1=bqk_sb[:, 5:6]
                )

                # v/vg projections (token-major, with bias broadcast add)
                for tt in range(TT):
                    vp = pps.tile([P, 384], f32, tag="vps")
                    for kt in range(FT):
                        nc.tensor.matmul(
                            out=vp[:],
                            lhsT=xT_sb[:, kt, tt * P : (tt + 1) * P],
                            rhs=wv_sb[:, kt, :],
                            start=(kt == 0),
                            stop=(kt == FT - 1),
                        )
                    nc.vector.tensor_tensor(
                        out=v_sb[:, tt, :, 0:DH],
                        in0=vp[:],
                        in1=bvb_sb[:],
                        op=OP.add,
                    )

            # operand views (each matmul operand pair shares a base partition)
            qv = [blk_sb[0:DH, 0, :], blk_sb[DH:P, 0, :], blk_sb[0:DH, 2, :]]
            kv = [blk_sb[0:DH, 1, :], blk_sb[DH:P, 1, :], blk_sb[0:DH, 3, :]]
            qgv = [qg5_sb[0:DH, 0:G], qg5_sb[DH:P, 0:G], blk_sb[DH:P, 2, 0:G]]
            kgv = [blk_sb[0:DH, 4, :], blk_sb[DH:P, 4, :], blk_sb[DH:P, 3, :]]
            ctxdst = [ctx01_sb[0:DH, :], ctxh1_sb[:, :], ctx2_sb[:, :]]

            # ---------------- attention ----------------
            with tc.tile_pool(name="att_sb", bufs=14) as ap_, \
                 tc.tile_pool(name="att_sb2", bufs=4) as ap2, \
                 tc.tile_pool(name="sc_ps", bufs=4, space="PSUM") as sps, \
                 tc.tile_pool(name="ctx_ps", bufs=2, space="PSUM") as cps, \
                 tc.tile_pool(name="bc_ps", bufs=2, space="PSUM") as bps2:

                def emit_pv(h, c, pg_t, ptiles):
                    """PV + denominator + normalize for (h, c)."""
                    cpsum = cps.tile([DH + 1, QW], f32, tag="cps")
                    nc.tensor.matmul(
                        out=cpsum[:],
                        lhsT=v_sb[0:G, 0, h, :],
                        rhs=pg_t[:],
                        start=True,
                        stop=False,
                    )
                    for idx, (jt, p_t) in enumerate(ptiles):
                        g_tt = 2 * (c - 1) + jt
                        nc.tensor.matmul(
                            out=cpsum[:],
                            lhsT=v_sb[:, g_tt, h, :],
                            rhs=p_t[:],
                            start=False,
                            stop=(idx == len(ptiles) - 1),
                        )
                    # denominator row -> reciprocal -> broadcast across partitions
                    rc = ap2.tile([P, QW], f32, tag="rc")
                    nc.vector.reciprocal(
                        out=rc[DH : DH + 1, :], in_=cpsum[DH : DH + 1, :]
                    )
                    bc = bps2.tile([DH, QW], f32, tag="bcp")
                    nc.tensor.matmul(
                        out=bc[:],
                        lhsT=ones_sb[DH : DH + 1, 0:DH],
                        rhs=rc[DH : DH + 1, :],
                        start=True,
                        stop=True,
                    )
                    bcs = ap2.tile([DH, QW], f32, tag="bcs")
                    nc.vector.tensor_copy(out=bcs[:], in_=bc[:])
                    nc.vector.tensor_tensor(
                        out=ctxdst[h][:, c * QW : (c + 1) * QW],
                        in0=cpsum[0:DH, :],
                        in1=bcs[:],
                        op=OP.mult,
                    )

                for h in range(HPC):
                    pending = None
                    for c in range(C):
                        # scores, transposed: key position on partitions
                        ptiles = []
                        for jt, mi in _chunk_schedule(c):
                            tok0 = (c - 1) * 2 * P + jt * P
                            sp = sps.tile([P, QW], f32, tag="sps")
                            nc.tensor.matmul(
                                out=sp[:],
                                lhsT=kv[h][:, tok0 : tok0 + P],
                                rhs=qv[h][:, c * QW : (c + 1) * QW],
                                start=True,
                                stop=True,
                            )
                            if mi is None:
                                p_t = ap_.tile([P, QW], bf16, tag="p")
                                nc.scalar.activation(p_t[:], sp[:], AF.Exp)
                            else:
                                pe_t = ap2.tile([P, QW], bf16, tag="pe")
                                nc.scalar.activation(pe_t[:], sp[:], AF.Exp)
                                p_t = ap_.tile([P, QW], bf16, tag="p")
                                nc.vector.tensor_tensor(
                                    out=p_t[:],
                                    in0=pe_t[:],
                                    in1=msk_sb[:, mi, :],
                                    op=OP.mult,
                                )
                            ptiles.append((jt, p_t))
                        # scores vs the G global keys (always unmasked)
                        sg = sps.tile([P, QW], f32, tag="sps")
                        nc.tensor.matmul(
                            out=sg[0:G, :],
                            lhsT=kv[h][:, 0:G],
                            rhs=qv[h][:, c * QW : (c + 1) * QW],
                            start=True,
                            stop=True,
                        )
                        pg_t = ap2.tile([G, QW], bf16, tag="pg")
                        nc.scalar.activation(pg_t[:], sg[0:G, :], AF.Exp)

                        if pending is not None:
                            emit_pv(h, pending[0], pending[1], pending[2])
                        pending = (c, pg_t, ptiles)
                    emit_pv(h, pending[0], pending[1], pending[2])

                # ---------------- global query rows ----------------
                for h in range(HPC):
                    for tb in range(TT // 4):
                        gp_ps = sps.tile([P, QW], f32, tag="sps")
                        for k in range(4):
                            tt = tb * 4 + k
                            nc.tensor.matmul(
                                out=gp_ps[:, k * G : (k + 1) * G],
                                lhsT=kgv[h][:, tt * P : (tt + 1) * P],
                                rhs=qgv[h][:],
                                start=True,
                                stop=True,
                            )
                        nc.scalar.activation(
                            pg_sb[:, tb * 4 : (tb + 1) * 4, h * G : (h + 1) * G],
                            gp_ps[:, 0 : 4 * G],
                            AF.Exp,
                        )
                    gc_ps = cps.tile([DH + 1, QW], f32, tag="cps")
                    for tt in range(TT):
                        nc.tensor.matmul(
                            out=gc_ps[:, 0:G],
                            lhsT=v_sb[:, tt, HPC + h, :],
                            rhs=pg_sb[:, tt, h * G : (h + 1) * G],
                            start=(tt == 0),
                            stop=(tt == TT - 1),
                        )
                    rcg = ap2.tile([P, QW], f32, tag="rc")
                    nc.vector.reciprocal(
                        out=rcg[DH : DH + 1, 0:G], in_=gc_ps[DH : DH + 1, 0:G]
                    )
                    bcg = bps2.tile([DH, QW], f32, tag="bcp")
                    nc.tensor.matmul(
                        out=bcg[:, 0:G],
                        lhsT=ones_sb[DH : DH + 1, 0:DH],
                        rhs=rcg[DH : DH + 1, 0:G],
                        start=True,
                        stop=True,
                    )
                    bcgs = ap2.tile([DH, QW], f32, tag="bcs")
                    nc.vector.tensor_copy(out=bcgs[:, 0:G], in_=bcg[:, 0:G])
                    nc.vector.tensor_tensor(
                        out=ctxdst[h][:, 0:G],
                        in0=gc_ps[0:DH, 0:G],
                        in1=bcgs[:, 0:G],
                        op=OP.mult,
                    )

                # head 1 ctx lives at base partition 0; move to rows 64:128
                nc.sync.dma_start(out=ctx01_sb[DH:P, :], in_=ctxh1_sb[:])

            # ---------------- output projection ----------------
            with tc.tile_pool(name="out_sb", bufs=3) as osb, \
                 tc.tile_pool(name="out_ps", bufs=2, space="PSUM") as ops:
                for tt in range(TT):
                    op_ps = ops.tile([P, D], f32, tag="ops")
                    for (n0, n1) in ((0, 512), (512, 768)):
                        nc.tensor.matmul(
                            out=op_ps[:, n0:n1],
                            lhsT=ctx01_sb[:, tt * P : (tt + 1) * P],
                            rhs=wo0_sb[:, n0:n1],
                            start=True,
                            stop=False,
                        )
                        nc.tensor.matmul(
                            out=op_ps[:, n0:n1],
                            lhsT=ctx2_sb[:, tt * P : (tt + 1) * P],
                            rhs=wo1_sb[:, n0:n1],
                            start=False,
                            stop=True,
                        )
                    ot = osb.tile([P, D], f32, tag="ot")
                    nc.vector.tensor_tensor(
                        out=ot[:], in0=op_ps[:], in1=bob_sb[:], op=OP.add
                    )
                    nc.sync.dma_start(
                        out=outp.ap()[tt * P : (tt + 1) * P, :], in_=ot[:]
                    )

    return nc


def _prep_core_inputs(core, input_ids, emb, Wq, bq, Wk, bk, Wv, bv,
                      Wqg, bqg, Wkg, bkg, Wvg, bvg, Wo, bo):
    b, hg = divmod(core, 4)
    hs = HPC * hg * DH           # feature offset of this core's head slice
    sl = slice(hs, hs + HPC * DH)

    def hcol(Wm, h):
        return np.asarray(Wm[:, hs + h * DH : hs + (h + 1) * DH], np.float32)

    def hbias(bm, h):
        return np.asarray(bm[hs + h * DH : hs + (h + 1) * DH], np.float32)

    # blocks: 0:(q0|q1) 1:(k0|k1) 2:(q2|qg2) 3:(k2|kg2) 4:(kg0|kg1) 5:(qg0|qg1)
    wq = [hcol(Wq, h) * SCALE for h in range(HPC)]
    wk = [hcol(Wk, h) for h in range(HPC)]
    wqg = [hcol(Wqg, h) * SCALE for h in range(HPC)]
    wkg = [hcol(Wkg, h) for h in range(HPC)]
    bq_ = [hbias(bq, h) * SCALE for h in range(HPC)]
    bk_ = [hbias(bk, h) for h in range(HPC)]
    bqg_ = [hbias(bqg, h) * SCALE for h in range(HPC)]
    bkg_ = [hbias(bkg, h) for h in range(HPC)]

    wqk_cat = np.concatenate(
        [wq[0], wq[1], wk[0], wk[1], wq[2], wqg[2], wk[2], wkg[2],
         wkg[0], wkg[1], wqg[0], wqg[1]], axis=1)
    bqk_cat = np.concatenate(
        [bq_[0], bq_[1], bk_[0], bk_[1], bq_[2], bqg_[2], bk_[2], bkg_[2],
         bkg_[0], bkg_[1], bqg_[0], bqg_[1]])

    wv_cat = np.concatenate(
        [hcol(Wv, h) for h in range(HPC)] + [hcol(Wvg, h) for h in range(HPC)],
        axis=1)
    bv_cat = np.concatenate(
        [hbias(bv, h) for h in range(HPC)] + [hbias(bvg, h) for h in range(HPC)])

    wo_cat = np.asarray(Wo[sl, :], np.float32)
    bo_in = np.asarray(bo, np.float32) if hg == 0 else np.zeros(
        (D,), np.float32)

    return {
        "ids": np.asarray(input_ids[b], np.int32).reshape(S, 1),
        "emb": np.ascontiguousarray(np.asarray(emb, np.float32)),
        "wqk": wqk_cat.astype(BF16),
        "bqk": bqk_cat.astype(np.float32),
        "wv": wv_cat.astype(BF16),
        "bv": bv_cat.astype(np.float32),
        "wo0": np.ascontiguousarray(wo_cat[0:P, :]).astype(BF16),
        "wo1": np.ascontiguousarray(wo_cat[P : P + DH, :]).astype(BF16),
        "bo": bo_in,
        "msk": _build_masks(),
    }


def kernel(**inputs):
    _install_axon_hooks()
    from concourse.bass_utils import run_bass_kernel_spmd

    if "nc" not in _COMPILED:
        _COMPILED["nc"] = build_nc()
    nc = _COMPILED["nc"]

    in_maps = [_prep_core_inputs(core, **inputs) for core in range(NCORES)]
    trace = bool(int(os.environ.get("KERNEL_TRACE", "0")))
    res = run_bass_kernel_spmd(nc, in_maps, list(range(NCORES)), trace=trace)
    _COMPILED["last_result"] = res

    out = np.zeros((B, S, D), np.float32)
    for core in range(NCORES):
        out[core // 4] += res.results[core]["out"]
    return out


# revision 10
# speedup vs baseline: 1.9312x; 1.9312x over previous
"""Longformer-style sparse attention block (nn_BasicNetwork_22892175688067).

Full-input contract: kernel(**inputs) takes the unsharded inputs and returns
the full [B, S, D] fp32 output.  Internally the (batch, head) grid is sharded
across 8 NeuronCores: core = b*4 + hg owns batch b and heads [3*hg, 3*hg+3).
Each core:
  - gathers x[b] = emb[ids[b]] on-device (indirect DMA),
  - projects q/k/v (+ global-token projections) for its 3 heads in bf16,
  - runs banded window attention + global tokens,
  - applies the output projection slice -> a partial [S, D] fp32 output.
The host sums the 4 per-batch partials (tensor-parallel unshard).
"""

import os
import sys
import types

import numpy as np

for _p in ("/opt/trn_rl_repo",):
    if os.path.isdir(_p) and _p not in sys.path:
        sys.path.insert(0, _p)

import ml_dtypes

BF16 = ml_dtypes.bfloat16

B, S, D = 2, 4096, 768
H, DH, W, G, VOCAB = 12, 64, 256, 16, 50265
HPC = 3               # heads per core
C = S // W            # 16 chunks
QW = W                # queries per chunk (= one-sided window)
P = 128
FT = D // P           # 6 feature k-tiles
TT = S // P           # 32 token tiles
NT = S // 512         # 8 n-tiles for qk projection
NCORES = 8
SCALE = 1.0 / 8.0     # 1/sqrt(DH), exact power of two

_COMPILED = {}


def _chunk_schedule(c):
    """Per q-chunk list of (window j-tile, mask id or None).

    mask ids: 0=M_A (j>=i), 1=M_A1 (j>=i & j>=16), 2=M_B (128+j>=i),
              3=M_C (j<=i), 4=M_D (j<=i-128), 5=M_E (j>=16)
    """
    if c == 0:
        return [(2, 5), (3, None), (4, 3), (5, 4)]
    if c == 1:
        return [(0, 1), (1, 2), (2, None), (3, None), (4, 3), (5, 4)]
    if c == C - 1:
        return [(0, 0), (1, 2), (2, None), (3, None)]
    return [(0, 0), (1, 2), (2, None), (3, None), (4, 3), (5, 4)]


def _build_masks():
    j = np.arange(P)[:, None]
    i = np.arange(QW)[None, :]
    masks = np.zeros((6, P, QW), np.float32)
    masks[0] = (j >= i)
    masks[1] = (j >= i) & (j >= G)
    masks[2] = (P + j >= i)
    masks[3] = (j <= i)
    masks[4] = (j <= i - P)
    masks[5] = (j >= G) * np.ones((1, QW))
    return masks.astype(BF16)


def _install_axon_hooks():
    """Provide antenv.axon_hooks (missing in this image) so NTFF tracing works."""
    if "antenv.axon_hooks" in sys.modules:
        return
    mod = types.ModuleType("antenv.axon_hooks")
    hook = [None]
    mod.set_axon_ntff_profile_hook = lambda h: hook.__setitem__(0, h)
    mod.get_axon_ntff_profile_hook = lambda: hook[0]
    sys.modules["antenv.axon_hooks"] = mod
    try:
        import antenv

        antenv.axon_hooks = mod
        from trn_agent_boot.trn_boot import _ntff_profile_via_ctypes

        mod.set_axon_ntff_profile_hook(
            _ntff_profile_via_ctypes("/opt/axon/libaxon_pjrt.so")
        )
    except Exception:
        pass


def _patch_tile_drain():
    """This walrus build allows only ONE sync-wait per instruction.

    Split extra waits onto same-engine NoOps emitted just before the
    instruction (engines execute their stream in order, so chained
    single-wait nops are semantically identical to one multi-wait inst).
    """
    import concourse.mybir as mybir
    import concourse.tile as tile
    from concourse.vector_clock import ScopedClock

    if getattr(tile.TileContext, "_drain_split_patched", False):
        return

    _orig_add = tile.TileContext._add_instruction
    counter = [0]

    def _add_instruction(self, inst):
        si = getattr(inst, "sync_info", None)
        if si is not None and si.on_wait is not None and len(si.on_wait) > 1:
            waits = list(si.on_wait)
            for w in waits[:-1]:
                counter[0] += 1
                nop = mybir.InstNoOp(
                    name=f"WS-{counter[0]}", text_hint="wait_split"
                )
                nop.engine = inst.engine
                nop.sync_info = mybir.SyncInfo(on_wait=[w], on_update=[])
                _orig_add(self, nop)
            inst.sync_info = mybir.SyncInfo(
                on_wait=[waits[-1]], on_update=list(si.on_update)
            )
        _orig_add(self, inst)

    tile.TileContext._add_instruction = _add_instruction

    def _drain_and_barrier(self, tick_clock, wait_clock):
        drain1 = self.nc.sync.drain()
        wait_clock.add_sem_waits(
            drain1.ins, ScopedClock({None: tick_clock.global_clock})
        )
        si = drain1.ins.sync_info
        waits = list(si.on_wait) if si is not None and si.on_wait else []
        if len(waits) > 1:
            drain1.ins.sync_info = mybir.SyncInfo(
                on_wait=waits[:1], on_update=list(si.on_update)
            )
            for wchunk in waits[1:]:
                d = self.nc.sync.drain()
                d.ins.sync_info = mybir.SyncInfo(on_wait=[wchunk], on_update=[])
        self.nc.all_engine_barrier()
        assert self.sems is not None
        popped = self.nc._tile_sem_poison_stack.pop()
        assert popped is self._sem_poison
        self.nc.clear_and_free_semaphores(list(self.sems.allocated().values()))
        self.nc.all_engine_barrier()

    tile.TileContext._drain_and_barrier = _drain_and_barrier
    tile.TileContext._drain_split_patched = True


def build_nc():
    """Build the per-core Bass program (identical on all 8 cores)."""
    import concourse.bass as bass
    import concourse.mybir as mybir
    import concourse.tile as tile

    _patch_tile_drain()

    f32 = mybir.dt.float32
    bf16 = mybir.dt.bfloat16
    i32 = mybir.dt.int32
    AF = mybir.ActivationFunctionType
    OP = mybir.AluOpType

    nc = bass.Bass("TRN2", num_devices=NCORES)

    ids = nc.dram_tensor("ids", [S, 1], i32, kind="ExternalInput")
    emb = nc.dram_tensor("emb", [VOCAB, D], f32, kind="ExternalInput")
    wqk = nc.dram_tensor("wqk", [D, 768], bf16, kind="ExternalInput")
    bqk = nc.dram_tensor("bqk", [768], f32, kind="ExternalInput")
    wv = nc.dram_tensor("wv", [D, 384], bf16, kind="ExternalInput")
    bv = nc.dram_tensor("bv", [384], f32, kind="ExternalInput")
    wo0 = nc.dram_tensor("wo0", [P, D], bf16, kind="ExternalInput")
    wo1 = nc.dram_tensor("wo1", [DH, D], bf16, kind="ExternalInput")
    bo = nc.dram_tensor("bo", [D], f32, kind="ExternalInput")
    msk = nc.dram_tensor("msk", [6, P, QW], bf16, kind="ExternalInput")
    outp = nc.dram_tensor("out", [S, D], f32, kind="ExternalOutput")

    with tile.TileContext(nc) as tc:
        from contextlib import ExitStack

        with ExitStack() as ctx:
            pers = ctx.enter_context(tc.tile_pool(name="pers", bufs=1))

            # ---------------- persistent SBUF tensors ----------------
            wqk_sb = pers.tile([P, FT, 768], bf16, tag="wqk")
            bqk_sb = pers.tile([P, FT], f32, tag="bqk")
            wv_sb = pers.tile([P, FT, 384], bf16, tag="wv")
            wo0_sb = pers.tile([P, D], bf16, tag="wo0")
            wo1_sb = pers.tile([DH, D], bf16, tag="wo1")
            msk_sb = pers.tile([P, 6, QW], bf16, tag="msk")
            ones_sb = pers.tile([P, P], f32, tag="ones")
            bv1_sb = pers.tile([1, 384], f32, tag="bv1")
            bo1_sb = pers.tile([1, D], f32, tag="bo1")
            bvb_sb = pers.tile([P, 384], f32, tag="bvb")
            bob_sb = pers.tile([P, D], f32, tag="bob")
            # projection outputs
            blk_sb = pers.tile([P, 5, S], bf16, tag="blk")
            qg5_sb = pers.tile([P, P], bf16, tag="qg5")
            v_sb = pers.tile([P, TT, 2 * HPC, DH + 1], bf16, tag="vsb")
            ctx01_sb = pers.tile([P, S], bf16, tag="ctx01")
            ctx2_sb = pers.tile([DH, S], bf16, tag="ctx2")
            ctxh1_sb = pers.tile([DH, S], bf16, tag="ctxh1")
            pg_sb = pers.tile([P, TT, HPC * G], bf16, tag="pgsb")

            # ---------------- constant loads ----------------
            nc.sync.dma_start(
                out=wqk_sb[:], in_=wqk.ap().rearrange("(kt p) c -> p kt c", p=P)
            )
            nc.sync.dma_start(
                out=bqk_sb[:], in_=bqk.ap().rearrange("(kt p) -> p kt", p=P)
            )
            nc.sync.dma_start(
                out=wv_sb[:], in_=wv.ap().rearrange("(kt p) c -> p kt c", p=P)
            )
            nc.sync.dma_start(out=wo0_sb[:], in_=wo0.ap())
            nc.sync.dma_start(out=wo1_sb[:], in_=wo1.ap())
            nc.sync.dma_start(out=msk_sb[:], in_=msk.ap().rearrange("m p i -> p m i"))
            nc.sync.dma_start(out=bv1_sb[:], in_=bv.ap()[None, :])
            nc.sync.dma_start(out=bo1_sb[:], in_=bo.ap()[None, :])
            nc.gpsimd.memset(ones_sb[:], 1.0)
            nc.gpsimd.memset(v_sb[:, :, :, DH], 1.0)

            # broadcast biases across partitions via ones-matmul (fp32)
            with tc.tile_pool(name="bcast_ps", bufs=1, space="PSUM") as bps:
                bvp = bps.tile([P, 384], f32, tag="bvp")
                nc.tensor.matmul(
                    out=bvp[:], lhsT=ones_sb[0:1, :], rhs=bv1_sb[:],
                    start=True, stop=True,
                )
                nc.vector.tensor_copy(out=bvb_sb[:], in_=bvp[:])
                bop = bps.tile([P, D], f32, tag="bop")
                nc.tensor.matmul(
                    out=bop[:, 0:512], lhsT=ones_sb[0:1, :], rhs=bo1_sb[:, 0:512],
                    start=True, stop=True,
                )
                nc.tensor.matmul(
                    out=bop[:, 512:768], lhsT=ones_sb[0:1, :], rhs=bo1_sb[:, 512:768],
                    start=True, stop=True,
                )
                nc.vector.tensor_copy(out=bob_sb[:], in_=bop[:])

            # ---------------- gather + transpose + projections ----------------
            from concourse.masks import make_identity

            ident_sb = pers.tile([P, P], f32, tag="ident")
            make_identity(nc, ident_sb[:])

            with tc.tile_pool(name="xt_pool", bufs=1) as xtp, \
                 tc.tile_pool(name="gather", bufs=3) as gp, \
                 tc.tile_pool(name="tp_ps", bufs=3, space="PSUM") as tpp, \
                 tc.tile_pool(name="proj_ps", bufs=2, space="PSUM") as pps:
                xT_sb = xtp.tile([P, FT, S], bf16, tag="xT")

                for tt in range(TT):
                    idx_t = gp.tile([P, 1], i32, tag="idx")
                    nc.sync.dma_start(
                        out=idx_t[:], in_=ids.ap()[tt * P : (tt + 1) * P, :]
                    )
                    xg = gp.tile([P, D], f32, tag="xg")
                    nc.gpsimd.indirect_dma_start(
                        out=xg[:],
                        out_offset=None,
                        in_=emb.ap(),
                        in_offset=bass.IndirectOffsetOnAxis(ap=idx_t[:, :1], axis=0),
                    )
                    for ft in range(FT):
                        tp = tpp.tile([P, P], f32, tag="tp")
                        nc.tensor.transpose(
                            out=tp[:],
                            in_=xg[:, ft * P : (ft + 1) * P],
                            identity=ident_sb[:],
                        )
                        dst = xT_sb[:, ft, tt * P : (tt + 1) * P]
                        if ft % 2 == 0:
                            nc.vector.tensor_copy(out=dst, in_=tp[:])
                        else:
                            nc.scalar.copy(out=dst, in_=tp[:])

                # q/k/qg/kg projections (feature-major outputs).
                # block cols: 0:(q0|q1) 1:(k0|k1) 2:(q2|qg2) 3:(k2|kg2) 4:(kg0|kg1)
                # B5 (qg0|qg1) done over token-tile 0 only -> qg5_sb
                for n in range(NT):
                    for bi in range(5):
                        ps = pps.tile([P, 512], f32, tag="pps")
                        for kt in range(FT):
                            nc.tensor.matmul(
                                out=ps[:],
                                lhsT=wqk_sb[:, kt, bi * P : (bi + 1) * P],
                                rhs=xT_sb[:, kt, n * 512 : (n + 1) * 512],
                                start=(kt == 0),
                                stop=(kt == FT - 1),
                            )
                        nc.vector.tensor_scalar_add(
                            out=blk_sb[:, bi, n * 512 : (n + 1) * 512],
                            in0=ps[:],
                            scalar1=bqk_sb[:, bi : bi + 1],
                        )
                ps5 = pps.tile([P, 512], f32, tag="pps")
                for kt in range(FT):
                    nc.tensor.matmul(
                        out=ps5[:, 0:P],
                        lhsT=wqk_sb[:, kt, 5 * P : 6 * P],
                        rhs=xT_sb[:, kt, 0:P],
                        start=(kt == 0),
                        stop=(kt == FT - 1),
                    )
                nc.vector.tensor_scalar_add(
                    out=qg5_sb[:], in0=ps5[:, 0:P], scalar1=bqk_sb[:, 5:6]
                )

                # v/vg projections (token-major, with bias broadcast add)
                for tt in range(TT):
                    vp = pps.tile([P, 384], f32, tag="vps")
                    for kt in range(FT):
                        nc.tensor.matmul(
                            out=vp[:],
                            lhsT=xT_sb[:, kt, tt * P : (tt + 1) * P],
                            rhs=wv_sb[:, kt, :],
                            start=(kt == 0),
                            stop=(kt == FT - 1),
                        )
                    nc.vector.tensor_tensor(
                        out=v_sb[:, tt, :, 0:DH],
                        in0=vp[:],
                        in1=bvb_sb[:],
                        op=OP.add,
                    )

            # operand views (each matmul operand pair shares a base partition)
            qv = [blk_sb[0:DH, 0, :], blk_sb[DH:P, 0, :], blk_sb[0:DH, 2, :]]
            kv = [blk_sb[0:DH, 1, :], blk_sb[DH:P, 1, :], blk_sb[0:DH, 3, :]]
            qgv = [qg5_sb[0:DH, 0:G], qg5_sb[DH:P, 0:G], blk_sb[DH:P, 2, 0:G]]
            kgv = [blk_sb[0:DH, 4, :], blk_sb[DH:P, 4, :], blk_sb[DH:P, 3, :]]
            ctxdst = [ctx01_sb[0:DH, :], ctxh1_sb[:, :], ctx2_sb[:, :]]

            # ---------------- attention ----------------
            with tc.tile_pool(name="att_sb", bufs=14) as ap_, \
                 tc.tile_pool(name="att_sb2", bufs=4) as ap2, \
                 tc.tile_pool(name="sc_ps", bufs=4, space="PSUM") as sps, \
                 tc.tile_pool(name="ctx_ps", bufs=2, space="PSUM") as cps, \
                 tc.tile_pool(name="bc_ps", bufs=2, space="PSUM") as bps2:

                def emit_pv(h, c, pg_t, ptiles):
                    """PV + denominator + normalize for (h, c)."""
                    cpsum = cps.tile([DH + 1, QW], f32, tag="cps")
                    nc.tensor.matmul(
                        out=cpsum[:],
                        lhsT=v_sb[0:G, 0, h, :],
                        rhs=pg_t[:],
                        start=True,
                        stop=False,
                    )
                    for idx, (jt, p_t, x0, x1) in enumerate(ptiles):
                        g_tt = 2 * (c - 1) + jt
                        nc.tensor.matmul(
                            out=cpsum[:, x0:x1],
                            lhsT=v_sb[:, g_tt, h, :],
                            rhs=p_t[:, x0:x1],
                            start=False,
                            stop=(idx == len(ptiles) - 1),
                        )
                    # denominator row -> 1/x as exp(-ln(x)) on ACT -> broadcast
                    ln_t = ap2.tile([P, QW], f32, tag="ln")
                    nc.scalar.activation(
                        ln_t[DH : DH + 1, :], cpsum[DH : DH + 1, :], AF.Ln
                    )
                    rc = ap2.tile([P, QW], f32, tag="rc")
                    nc.scalar.activation(
                        rc[DH : DH + 1, :], ln_t[DH : DH + 1, :], AF.Exp, scale=-1.0
                    )
                    bc = bps2.tile([DH, QW], f32, tag="bcp")
                    nc.tensor.matmul(
                        out=bc[:],
                        lhsT=ones_sb[DH : DH + 1, 0:DH],
                        rhs=rc[DH : DH + 1, :],
                        start=True,
                        stop=True,
                    )
                    bcs = ap2.tile([DH, QW], f32, tag="bcs")
                    nc.vector.tensor_copy(out=bcs[:], in_=bc[:])
                    nc.vector.tensor_tensor(
                        out=ctxdst[h][:, c * QW : (c + 1) * QW],
                        in0=cpsum[0:DH, :],
                        in1=bcs[:],
                        op=OP.mult,
                    )

                for h in range(HPC):
                    pending = None
                    for c in range(C):
                        # scores, transposed: key position on partitions
                        ptiles = []
                        for jt, mi in _chunk_schedule(c):
                            tok0 = (c - 1) * 2 * P + jt * P
                            # edge j-tiles are half-masked: only compute the
                            # live query half (jt0 -> i<128, jt5 -> i>=128)
                            if jt == 0:
                                x0, x1 = 0, P
                            elif jt == 5:
                                x0, x1 = P, QW
                            else:
                                x0, x1 = 0, QW
                            sp = sps.tile([P, QW], f32, tag="sps")
                            nc.tensor.matmul(
                                out=sp[:, x0:x1],
                                lhsT=kv[h][:, tok0 : tok0 + P],
                                rhs=qv[h][:, c * QW + x0 : c * QW + x1],
                                start=True,
                                stop=True,
                            )
                            p_t = ap_.tile([P, QW], bf16, tag="p")
                            if mi is None:
                                nc.scalar.activation(
                                    p_t[:, x0:x1], sp[:, x0:x1], AF.Exp
                                )
                            else:
                                pe_t = ap2.tile([P, QW], bf16, tag="pe")
                                nc.scalar.activation(
                                    pe_t[:, x0:x1], sp[:, x0:x1], AF.Exp
                                )
                                mm_eng = nc.gpsimd if x1 - x0 == QW else nc.vector
                                mm_eng.tensor_tensor(
                                    out=p_t[:, x0:x1],
                                    in0=pe_t[:, x0:x1],
                                    in1=msk_sb[:, mi, x0:x1],
                                    op=OP.mult,
                                )
                            ptiles.append((jt, p_t, x0, x1))
                        # scores vs the G global keys (always unmasked)
                        sg = sps.tile([P, QW], f32, tag="sps")
                        nc.tensor.matmul(
                            out=sg[0:G, :],
                            lhsT=kv[h][:, 0:G],
                            rhs=qv[h][:, c * QW : (c + 1) * QW],
                            start=True,
                            stop=True,
                        )
                        pg_t = ap2.tile([G, QW], bf16, tag="pg")
                        nc.scalar.activation(pg_t[:], sg[0:G, :], AF.Exp)

                        if pending is not None:
                            emit_pv(h, pending[0], pending[1], pending[2])
                        pending = (c, pg_t, ptiles)
                    emit_pv(h, pending[0], pending[1], pending[2])

                # ---------------- global query rows ----------------
                for h in range(HPC):
                    for tb in range(TT // 4):
                        gp_ps = sps.tile([P, QW], f32, tag="sps")
                        for k in range(4):
                            tt = tb * 4 + k
                            nc.tensor.matmul(
                                out=gp_ps[:, k * G : (k + 1) * G],
                                lhsT=kgv[h][:, tt * P : (tt + 1) * P],
                                rhs=qgv[h][:],
                                start=True,
                                stop=True,
                            )
                        nc.scalar.activation(
                            pg_sb[:, tb * 4 : (tb + 1) * 4, h * G : (h + 1) * G],
                            gp_ps[:, 0 : 4 * G],
                            AF.Exp,
                        )
                    gc_ps = cps.tile([DH + 1, QW], f32, tag="cps")
                    for tt in range(TT):
                        nc.tensor.matmul(
                            out=gc_ps[:, 0:G],
                            lhsT=v_sb[:, tt, HPC + h, :],
                            rhs=pg_sb[:, tt, h * G : (h + 1) * G],
                            start=(tt == 0),
                            stop=(tt == TT - 1),
                        )
                    lng = ap2.tile([P, QW], f32, tag="ln")
                    nc.scalar.activation(
                        lng[DH : DH + 1, 0:G], gc_ps[DH : DH + 1, 0:G], AF.Ln
                    )
                    rcg = ap2.tile([P, QW], f32, tag="rc")
                    nc.scalar.activation(
                        rcg[DH : DH + 1, 0:G], lng[DH : DH + 1, 0:G], AF.Exp,
                        scale=-1.0,
                    )
                    bcg = bps2.tile([DH, QW], f32, tag="bcp")
                    nc.tensor.matmul(
                        out=bcg[:, 0:G],
                        lhsT=ones_sb[DH : DH + 1, 0:DH],
                        rhs=rcg[DH : DH + 1, 0:G],
                        start=True,
                        stop=True,
                    )
                    bcgs = ap2.tile([DH, QW], f32, tag="bcs")
                    nc.vector.tensor_copy(out=bcgs[:, 0:G], in_=bcg[:, 0:G])
                    nc.vector.tensor_tensor(
                        out=ctxdst[h][:, 0:G],
                        in0=gc_ps[0:DH, 0:G],
                        in1=bcgs[:, 0:G],
                        op=OP.mult,
                    )

                # head 1 ctx lives at base partition 0; move to rows 64:128
                nc.sync.dma_start(out=ctx01_sb[DH:P, :], in_=ctxh1_sb[:])

            # ---------------- output projection ----------------
            with tc.tile_pool(name="out_sb", bufs=3) as osb, \
                 tc.tile_pool(name="out_ps", bufs=2, space="PSUM") as ops:
                for tt in range(TT):
                    op_ps = ops.tile([P, D], f32, tag="ops")
                    for (n0, n1) in ((0, 512), (512, 768)):
                        nc.tensor.matmul(
                            out=op_ps[:, n0:n1],
                            lhsT=ctx01_sb[:, tt * P : (tt + 1) * P],
                            rhs=wo0_sb[:, n0:n1],
                            start=True,
                            stop=False,
                        )
                        nc.tensor.matmul(
                            out=op_ps[:, n0:n1],
                            lhsT=ctx2_sb[:, tt * P : (tt + 1) * P],
                            rhs=wo1_sb[:, n0:n1],
                            start=False,
                            stop=True,
                        )
                    ot = osb.tile([P, D], f32, tag="ot")
                    nc.vector.tensor_tensor(
                        out=ot[:], in0=op_ps[:], in1=bob_sb[:], op=OP.add
                    )
                    nc.sync.dma_start(
                        out=outp.ap()[tt * P : (tt + 1) * P, :], in_=ot[:]
                    )

    return nc


def _prep_core_inputs(core, input_ids, emb, Wq, bq, Wk, bk, Wv, bv,
                      Wqg, bqg, Wkg, bkg, Wvg, bvg, Wo, bo):
    b, hg = divmod(core, 4)
    hs = HPC * hg * DH           # feature offset of this core's head slice
    sl = slice(hs, hs + HPC * DH)

    def hcol(Wm, h):
        return np.asarray(Wm[:, hs + h * DH : hs + (h + 1) * DH], np.float32)

    def hbias(bm, h):
        return np.asarray(bm[hs + h * DH : hs + (h + 1) * DH], np.float32)

    # blocks: 0:(q0|q1) 1:(k0|k1) 2:(q2|qg2) 3:(k2|kg2) 4:(kg0|kg1) 5:(qg0|qg1)
    wq = [hcol(Wq, h) * SCALE for h in range(HPC)]
    wk = [hcol(Wk, h) for h in range(HPC)]
    wqg = [hcol(Wqg, h) * SCALE for h in range(HPC)]
    wkg = [hcol(Wkg, h) for h in range(HPC)]
    bq_ = [hbias(bq, h) * SCALE for h in range(HPC)]
    bk_ = [hbias(bk, h) for h in range(HPC)]
    bqg_ = [hbias(bqg, h) * SCALE for h in range(HPC)]
    bkg_ = [hbias(bkg, h) for h in range(HPC)]

    wqk_cat = np.concatenate(
        [wq[0], wq[1], wk[0], wk[1], wq[2], wqg[2], wk[2], wkg[2],
         wkg[0], wkg[1], wqg[0], wqg[1]], axis=1)
    bqk_cat = np.concatenate(
        [bq_[0], bq_[1], bk_[0], bk_[1], bq_[2], bqg_[2], bk_[2], bkg_[2],
         bkg_[0], bkg_[1], bqg_[0], bqg_[1]])

    wv_cat = np.concatenate(
        [hcol(Wv, h) for h in range(HPC)] + [hcol(Wvg, h) for h in range(HPC)],
        axis=1)
    bv_cat = np.concatenate(
        [hbias(bv, h) for h in range(HPC)] + [hbias(bvg, h) for h in range(HPC)])

    wo_cat = np.asarray(Wo[sl, :], np.float32)
    bo_in = np.asarray(bo, np.float32) if hg == 0 else np.zeros(
        (D,), np.float32)

    return {
        "ids": np.asarray(input_ids[b], np.int32).reshape(S, 1),
        "emb": np.ascontiguousarray(np.asarray(emb, np.float32)),
        "wqk": wqk_cat.astype(BF16),
        "bqk": bqk_cat.astype(np.float32),
        "wv": wv_cat.astype(BF16),
        "bv": bv_cat.astype(np.float32),
        "wo0": np.ascontiguousarray(wo_cat[0:P, :]).astype(BF16),
        "wo1": np.ascontiguousarray(wo_cat[P : P + DH, :]).astype(BF16),
        "bo": bo_in,
        "msk": _build_masks(),
    }


def kernel(**inputs):
    _install_axon_hooks()
    from concourse.bass_utils import run_bass_kernel_spmd

    if "nc" not in _COMPILED:
        _COMPILED["nc"] = build_nc()
    nc = _COMPILED["nc"]

    in_maps = [_prep_core_inputs(core, **inputs) for core in range(NCORES)]
    trace = bool(int(os.environ.get("KERNEL_TRACE", "0")))
    res = run_bass_kernel_spmd(nc, in_maps, list(range(NCORES)), trace=trace)
    _COMPILED["last_result"] = res

    out = np.zeros((B, S, D), np.float32)
    for core in range(NCORES):
        out[core // 4] += res.results[core]["out"]
    return out
